# revision 43
# baseline (speedup 1.0000x reference)
"""AttentionFusion kernel for 8 Trainium2 NeuronCores (fp8 DoubleRow version).

Reference computation (B=2, C=256, H=W=64, N=8192 tokens = 2 modalities x 4096):
    x    = concat(flat(feat0), flat(feat1))        # [B, N, C]
    Q,K,V = x @ W{q,k,v}.T + b{q,k,v}
    attn = softmax(Q @ K.T / 16)
    out  = (attn @ V) @ Wo.T + bo                  # [B, N, C]
    out  = mean over modalities -> [B, HW, C] -> [B, C, H, W]

Sharding: 8 cores = (2 batches) x (4 query groups). Core (b, g) computes
queries {g*1024..(g+1)*1024} of each modality (2048 rows) for batch b, with
full K/V (8192 tokens) computed locally. The modality mean pairs rows within
a core, so there is no cross-core communication at all.

Everything is computed in "transposed" (feature-on-partition) layout; see the
bf16 baseline (kernel_bf16_baseline.py) for the derivation.  This version:

  * attention matmuls (S = K'Q, O += V'P, sums += 1'P) run in fp8 e4m3 with
    perf_mode=DoubleRow: 256-deep contraction per matmul at 2 MAC/cell/cyc.
    kT8/qT8 pack the two 128-channel halves as DoubleRow pairs; P^T pairs two
    consecutive key tiles.
  * exp is computed once per key-tile *pair* ([128,1024] PSUM read) with the
    softmax shift folded into the ACT bias: p = exp(s - 2).  The shift keeps
    exp(s) < 240 (TRN e4m3 overflows to Inf above 240, max observed s=6.5)
    and cancels in the softmax.
  * softmax denominators accumulate on the PE via a DoubleRow ones-matmul
    into a [128,512] PSUM bank — the all-ones stationary broadcasts the sums
    to every partition for free (the DVE cannot keep up with 16.8M adds/core,
    and GPSIMD cannot read PSUM).
  * normalization is deferred past the output projection (diag(1/s) commutes
    with Wo): PSUM accumulators are released by plain copies so the next
    chunk never waits on the reciprocal (reciprocal_approx_fast, ~51 ULP).
  * X is pre-cast to fp16 on the host; all inputs arrive in 7 large DMAs
    split across the sync+ACT dispatch queues (each dma_start costs ~0.6us
    of serial engine dispatch time at startup).
  * 48 dependency-free warm-up matmuls (~5us) bring the PE HAM clock gate to
    8/8 before the first real matmul (below ~3.4us of activity the PE stays
    at 1.2GHz and phase 1 runs 2x slow).
  * chunk 0's whole attention pipeline is interleaved into the K/V
    production loop (single-k-tile S tiles + half-exps there), hiding the
    PE-bound projection phase under the ACT-bound attention phase; each
    chunk's projection tail is emitted inside the next chunk's pair loop.

Numerics: fp8 e4m3 Q/K/P/V (l2 rel err ~1.4e-2 vs the 2e-2 gate, dominated
by Q.K quantization), fp16 projections/weights, fp32 PSUM + softmax sums.
bk is dropped (softmax invariant); bv is folded into bo_eff = bo + Wo @ bv.
"""

import numpy as np

B, C, H, W = 2, 256, 64, 64
HW = H * W            # 4096
NTOK = 2 * HW         # 8192 tokens per batch (2 modalities)
NQ = 2048             # q columns per core
P = 128
KT = NTOK // P        # 64 k-tiles
PAIRS = KT // 2       # 32 k-tile pairs
QCH = 512             # q-chunk width
NCH = NQ // QCH       # 4 q-chunks per core
NCORES = 8

_compiled = {}


def _build():
    import concourse.bass as bass  # noqa: F401
    import concourse.mybir as mybir
    from concourse import bacc
    from concourse.tile import TileContext

    f32 = mybir.dt.float32
    f16 = mybir.dt.float16
    f8 = mybir.dt.float8e4
    COPY = mybir.ActivationFunctionType.Copy
    EXP = mybir.ActivationFunctionType.Exp
    DR = mybir.MatmulPerfMode.DoubleRow

    nc = bacc.Bacc("TRN2", target_bir_lowering=False, debug=False,
                   num_devices=NCORES)

    xT = nc.dram_tensor("xT", [C, NTOK], f16, kind="ExternalInput")
    xTq = nc.dram_tensor("xTq", [C, NQ], f16, kind="ExternalInput")
    # all 4 weight matrices packed into one [128, 8*256] DMA (row halves
    # h=0/1 of wq, wk, wv, wo) and both biases into one [128, 4] DMA —
    # each dma_start costs ~0.6us of serial engine dispatch at startup
    wpk_d = nc.dram_tensor("wpack", [P, 8 * C], f16, kind="ExternalInput")
    bpk_d = nc.dram_tensor("bpack", [P, 4], f32, kind="ExternalInput")
    out_d = nc.dram_tensor("out", [C, NQ // 2], f16, kind="ExternalOutput")

    with TileContext(nc) as tc:
        with tc.tile_pool(name="const", bufs=1) as cpool, \
             tc.tile_pool(name="kTp", bufs=1) as kTp, \
             tc.tile_pool(name="qTp", bufs=1) as qTp, \
             tc.tile_pool(name="Vp", bufs=1) as Vp:

            # all-ones stationary [128, 2, 128]: the sum-matmul broadcasts the
            # softmax denominators to all 128 output partitions for free
            # (pair-dim stride 128 satisfies the %16 LDWEIGHTS restriction)
            ones8 = cpool.tile([P, 2, P], f8, tag="ones8")
            nc.vector.memset(ones8[:], 1.0)
            ones8_3d = ones8[:, :, :]

            shift = cpool.tile([P, 1], f32, tag="shift")
            nc.vector.memset(shift[:], -2.0)
            # tiny dummy exp pulls the ~1.3us ACT_TABLE_LOAD into the
            # startup window instead of the first real exp
            dummy = cpool.tile([P, 1], f32, tag="dummy")
            nc.scalar.activation(dummy[:], shift[:], EXP)

            # ---- constants: packed fp16 weights + biases (2 DMAs on ACT,
            # concurrent with the X DMAs dispatched from sync) ----
            wtile = cpool.tile([P, 8 * C], f16, tag="wpack")
            nc.scalar.dma_start(wtile[:], wpk_d.ap())
            btile = cpool.tile([P, 4], f32, tag="bpack")
            nc.scalar.dma_start(btile[:], bpk_d.ap())
            wq16 = [wtile[:, 0 * C:1 * C], wtile[:, 1 * C:2 * C]]
            wk16 = [wtile[:, 2 * C:3 * C], wtile[:, 3 * C:4 * C]]
            wv16 = [wtile[:, 4 * C:5 * C], wtile[:, 5 * C:6 * C]]
            wo16 = [wtile[:, 6 * C:7 * C], wtile[:, 7 * C:8 * C]]
            bq_sb = [btile[:, 0:1], btile[:, 1:2]]
            bo_sb = [btile[:, 2:3], btile[:, 3:4]]

            # persistent fp8 activations (DoubleRow pair layouts)
            # kT8[p, ch, t] = K^T[ch*128+p, t]; qT8 same for Q^T/16
            kT8 = kTp.tile([P, 2 * NTOK], f8, tag="kT8", name="kT8")
            qT8 = qTp.tile([P, 2 * NQ], f8, tag="qT8", name="qT8")
            # Vb8[p, kt, c] = V[kt*128+p, c]
            Vb8 = Vp.tile([P, KT * C], f8, tag="Vb8", name="Vb8")
            kT8_3d = kT8[:].rearrange("p (two n) -> p two n", two=2)
            qT8_3d = qT8[:].rearrange("p (two n) -> p two n", two=2)
            Vb8_3d = Vb8[:].rearrange("p (kt c) -> p kt c", c=C)

            # ---- merged schedule: chunk 0's attention rides the K/V
            # production loop (phase 1 is PE-bound, phase 2 is ACT-bound,
            # so interleaving hides one under the other).  Outer pools hold
            # state that crosses the two regions; PSUM is exactly 8 banks
            # in each: region 1 = p1k(2)+p1v(1)+sp1(2)+o_ps(2)+sm(1),
            # region 2 = sps(4)+fp(1)+o_ps(2)+sm(1).
            with tc.tile_pool(name="ops", bufs=1, space="PSUM") as ops, \
                 tc.tile_pool(name="smp", bufs=1, space="PSUM") as smp, \
                 tc.tile_pool(name="pp", bufs=3) as pp, \
                 tc.tile_pool(name="ipp", bufs=2) as ipp, \
                 tc.tile_pool(name="bcp", bufs=2) as bcp, \
                 tc.tile_pool(name="nrm", bufs=2) as nrm, \
                 tc.tile_pool(name="tsp", bufs=6) as tsp, \
                 tc.tile_pool(name="osb", bufs=2) as osb:

                stash = {}
                deferred = []

                def make_flush(o_ps, sm):
                    def flush(item):
                        tp, p8c = item
                        first, last = tp == 0, tp == PAIRS - 1
                        p8c_3d = p8c[:].rearrange("p (two n) -> p two n",
                                                  two=2)
                        nc.tensor.matmul(sm[:], ones8_3d, p8c_3d,
                                         start=first, stop=last,
                                         perf_mode=DR)
                        for ch in range(2):
                            nc.tensor.matmul(
                                o_ps[ch][:],
                                Vb8_3d[:, 2 * tp:2 * tp + 2,
                                       ch * P:(ch + 1) * P],
                                p8c_3d, start=first, stop=last,
                                perf_mode=DR)
                    return flush

                def release_and_tail(chunk, o_ps, sm, fp_pool):
                    # free PSUM fast via plain copies; normalization is
                    # applied after the output projection (diag(1/s)
                    # commutes with Wo) so nothing waits on the reciprocal
                    last = chunk == 3
                    if last:
                        sm_src = sm
                    else:
                        sm_src = bcp.tile([P, QCH], f32, tag="sms",
                                          name=f"sms{chunk}")
                        nc.vector.tensor_scalar_add(sm_src[:], sm[:], 0.0)
                    n = []
                    for ch in range(2):
                        nt = nrm.tile([P, QCH], f16, tag="no",
                                      name=f"n{chunk}_{ch}")
                        nc.vector.tensor_scalar_add(nt[:], o_ps[ch][:], 0.0)
                        n.append(nt)

                    def tail(chunk=chunk, sm_src=sm_src, n=tuple(n)):
                        ip = ipp.tile([P, QCH], f32, tag="ip",
                                      name=f"ip{chunk}")
                        nc.vector.reciprocal_approx_fast(ip[:], sm_src[:])
                        for ch in range(2):
                            fp = fp_pool[0].tile([P, QCH], f32, tag="fp",
                                                 bufs=1, name=f"f{chunk}_{ch}")
                            nc.tensor.matmul(fp[:],
                                             wo16[0][:, ch * P:(ch + 1) * P],
                                             n[0][:], start=True, stop=False)
                            nc.tensor.matmul(fp[:],
                                             wo16[1][:, ch * P:(ch + 1) * P],
                                             n[1][:], start=False, stop=True)
                            t1 = nrm.tile([P, QCH], f16, tag="t1",
                                          name=f"u{chunk}_{ch}")
                            nc.vector.tensor_mul(t1[:], fp[:], ip[:])
                            if chunk < NCH // 2:
                                # stashed modality-0 half carries the full
                                # bias: tt = 0.5*t1 + (bo + Wo@bv)
                                tt = tsp.tile([P, QCH], f16, tag="ts",
                                              name=f"t{chunk}_{ch}")
                                nc.vector.tensor_scalar(
                                    tt[:], t1[:], 0.5, bo_sb[ch][:],
                                    mybir.AluOpType.mult, mybir.AluOpType.add)
                                stash.setdefault(chunk, []).append(tt)
                            else:
                                # paired modality-1 half: one fused op
                                # ot = 0.5*t1 + tt_stash, then stream out
                                prev = stash[chunk - NCH // 2][ch]
                                ob = (chunk - NCH // 2) * QCH
                                ot = osb.tile([P, QCH], f16, tag="os",
                                              name=f"ot{chunk}_{ch}")
                                nc.vector.scalar_tensor_tensor(
                                    ot[:], t1[:], 0.5, prev[:],
                                    mybir.AluOpType.mult,
                                    mybir.AluOpType.add)
                                nc.sync.dma_start(
                                    out_d.ap()[ch * P:(ch + 1) * P,
                                               ob:ob + QCH], ot[:])
                    return tail

                fp_pool = [None]  # bound to the region-2 S pool below

                # ---- region 1: phase 1 + chunk 0 interleaved ----
                with tc.tile_pool(name="xc", bufs=1) as xcp, \
                     tc.tile_pool(name="p1ps", bufs=2, space="PSUM") as p1k, \
                     tc.tile_pool(name="p1vs", bufs=1, space="PSUM") as p1v, \
                     tc.tile_pool(name="sp1", bufs=2, space="PSUM") as sp1:

                    # PE warm-up: ~5us of dependency-free matmuls so the HAM
                    # clock gate reaches 8/8 before the first real matmul
                    warm = p1k.tile([P, P], f32, tag="kp", name="warm")
                    for _ in range(48):
                        nc.tensor.matmul(warm[:], ones8_3d, ones8_3d,
                                         start=True, stop=True, perf_mode=DR)

                    # queries first (chunk 0 needs all of qT8);
                    # one DMA for all of xTq ([128, 2, 2048], h on dim 1)
                    xq2 = xTq.ap().rearrange("(two p) n -> p two n", two=2)
                    xqt = xcp.tile([P, 2, NQ], f16, tag="xq")
                    nc.sync.dma_start(xqt[:, :, :], xq2)
                    for j in range(NQ // 512):
                        for ch in range(2):
                            qp = p1k.tile([P, 512], f32, tag="kp",
                                          name=f"qp{ch}_{j}")
                            nc.tensor.matmul(qp[:],
                                             wq16[0][:, ch * P:(ch + 1) * P],
                                             xqt[:, 0, j * 512:(j + 1) * 512],
                                             start=True, stop=False)
                            nc.tensor.matmul(qp[:],
                                             wq16[1][:, ch * P:(ch + 1) * P],
                                             xqt[:, 1, j * 512:(j + 1) * 512],
                                             start=False, stop=True)
                            # qT8 = (Q^T + bq)/16 e4m3 (host: bq_eff=bq/16)
                            nc.vector.tensor_scalar(
                                qT8[:, ch * NQ + j * 512:
                                    ch * NQ + (j + 1) * 512],
                                qp[:], 1.0 / 16.0, bq_sb[ch][:],
                                mybir.AluOpType.mult, mybir.AluOpType.add)

                    o_ps0 = [ops.tile([P, QCH], f32, tag="op", bufs=2,
                                      name=f"o0_{h}") for h in range(2)]
                    sm0 = smp.tile([P, QCH], f32, tag="sm", name="sm0")
                    flush0 = make_flush(o_ps0, sm0)
                    rhs_q0 = qT8_3d[:, :, 0:QCH]
                    pending = []

                    # K/V inputs: 4 big DMAs of [128, 2, 2048] each; after
                    # each j-iter the two freshly produced k-tile pairs of
                    # chunk 0 are attended immediately
                    xk2 = xT.ap().rearrange("(two p) n -> p two n", two=2)
                    for jj in range(4):
                        xkt = xcp.tile([P, 2, 2048], f16, tag="xk", bufs=2,
                                       name=f"xk{jj}")
                        nc.sync.dma_start(
                            xkt[:, :, :], xk2[:, :, jj * 2048:(jj + 1) * 2048])
                        for j2 in range(4):
                            j = 4 * jj + j2
                            for ch in range(2):
                                kp = p1k.tile([P, 512], f32, tag="kp",
                                              name=f"kp{ch}_{j}")
                                nc.tensor.matmul(
                                    kp[:], wk16[0][:, ch * P:(ch + 1) * P],
                                    xkt[:, 0, j2 * 512:(j2 + 1) * 512],
                                    start=True, stop=False)
                                nc.tensor.matmul(
                                    kp[:], wk16[1][:, ch * P:(ch + 1) * P],
                                    xkt[:, 1, j2 * 512:(j2 + 1) * 512],
                                    start=False, stop=True)
                                dst = kT8[:, ch * NTOK + j * 512:
                                          ch * NTOK + (j + 1) * 512]
                                nc.vector.tensor_scalar_add(dst, kp[:], 0.0)
                            for u in range(2):  # 2 token-tiles per bank
                                vp = p1v.tile([P, 512], f32, tag="vp",
                                              name=f"vp{u}_{j}")
                                for t2 in range(2):
                                    t = j2 * 4 + 2 * u + t2
                                    nc.tensor.matmul(
                                        vp[:, t2 * C:(t2 + 1) * C],
                                        xkt[:, 0, t * P:(t + 1) * P],
                                        wv16[0][:], start=True, stop=False)
                                    nc.tensor.matmul(
                                        vp[:, t2 * C:(t2 + 1) * C],
                                        xkt[:, 1, t * P:(t + 1) * P],
                                        wv16[1][:], start=False, stop=True)
                                kt = 4 * j + 2 * u
                                nc.vector.tensor_scalar_add(
                                    Vb8[:, kt * C:(kt + 2) * C], vp[:], 0.0)
                            # chunk 0, pairs 2j and 2j+1 (k-tiles 4j..4j+3):
                            # single-k-tile S tiles (1 bank, double-buffered)
                            # with half-exps so S and exp overlap here too
                            for pr in (2 * j, 2 * j + 1):
                                p8t = pp.tile([P, 2 * QCH], f8, tag="p8",
                                              name=f"p0_{pr}")
                                for o in range(2):
                                    kt = 2 * pr + o
                                    ps = sp1.tile([P, QCH], f32, tag="sp",
                                                  name=f"s0_{kt}")
                                    nc.tensor.matmul(
                                        ps[:],
                                        kT8_3d[:, :, kt * P:(kt + 1) * P],
                                        rhs_q0, start=True, stop=True,
                                        perf_mode=DR)
                                    nc.scalar.activation(
                                        p8t[:, o * QCH:(o + 1) * QCH],
                                        ps[:], EXP, bias=shift[:])
                                pending.append((pr, p8t))
                                if len(pending) > 1:
                                    flush0(pending.pop(0))
                    flush0(pending.pop(0))
                    deferred.append(release_and_tail(0, o_ps0, sm0, fp_pool))
                # ---- region 2: chunks 2, 1, 3 (0 done above); modality-0
                # chunk's stash is consumed by its modality-1 partner ----
                with tc.tile_pool(name="sps", bufs=2, space="PSUM") as sps:
                    fp_pool[0] = sps
                    for chunk in (2, 1, 3):
                        qb = chunk * QCH
                        rhs_q = qT8_3d[:, :, qb:qb + QCH]
                        o_ps = [ops.tile([P, QCH], f32, tag="op", bufs=2,
                                         name=f"o{chunk}_{h}")
                                for h in range(2)]
                        sm = smp.tile([P, QCH], f32, tag="sm",
                                      name=f"sm{chunk}")
                        flush = make_flush(o_ps, sm)

                        # S-matmuls for pair t+1 are emitted before sum/O of
                        # pair t so the PE never waits on exp
                        pending = []
                        for t in range(PAIRS):
                            ps = sps.tile([P, 2 * QCH], f32, tag="sp",
                                          name=f"s{chunk}_{t}")
                            for o in range(2):
                                kt = 2 * t + o
                                nc.tensor.matmul(
                                    ps[:, o * QCH:(o + 1) * QCH],
                                    kT8_3d[:, :, kt * P:(kt + 1) * P],
                                    rhs_q, start=True, stop=True,
                                    perf_mode=DR)
                            p8t = pp.tile([P, 2 * QCH], f8, tag="p8",
                                          name=f"p{chunk}_{t}")
                            nc.scalar.activation(p8t[:], ps[:], EXP,
                                                 bias=shift[:])
                            pending.append((t, p8t))
                            if t == 2 and deferred:
                                # previous chunk's projection tail, emitted
                                # here so its matmuls never stall the PE at
                                # the chunk boundary (inputs ready by now)
                                deferred.pop(0)()
                            if len(pending) > 1:
                                flush(pending.pop(0))
                        flush(pending.pop(0))

                        tail = release_and_tail(chunk, o_ps, sm, fp_pool)
                        if chunk == 3:
                            while deferred:
                                deferred.pop(0)()
                            tail()
                        else:
                            deferred.append(tail)

    nc.compile()
    return nc


def _get_compiled():
    if "nc" not in _compiled:
        _compiled["nc"] = _build()
    return _compiled["nc"]


def kernel(feat0, feat1, Wq, bq, Wk, bk, Wv, bv, Wo, bo):
    from concourse.bass_utils import run_bass_kernel_spmd

    feat0 = np.asarray(feat0, dtype=np.float32)
    feat1 = np.asarray(feat1, dtype=np.float32)
    Wq = np.asarray(Wq, dtype=np.float32)
    Wk = np.asarray(Wk, dtype=np.float32)
    Wv = np.asarray(Wv, dtype=np.float32)
    Wo = np.asarray(Wo, dtype=np.float32)
    bq = np.asarray(bq, dtype=np.float32)
    bv = np.asarray(bv, dtype=np.float32)
    bo = np.asarray(bo, dtype=np.float32)

    wqT = Wq.T.astype(np.float16)
    wkT = Wk.T.astype(np.float16)
    wvT = Wv.T.astype(np.float16)
    woT = Wo.T.astype(np.float16)
    # row halves h=0/1 of each weight, packed side by side: [128, 8*256]
    wpack = np.ascontiguousarray(np.concatenate(
        [w[h * 128:(h + 1) * 128, :] for w in (wqT, wkT, wvT, woT)
         for h in range(2)], axis=1))
    bq_eff = bq / 16.0
    bo_eff = bo + Wo @ bv  # full bias, applied on the stash side
    bpack = np.ascontiguousarray(np.stack(
        [bq_eff[:128], bq_eff[128:], bo_eff[:128], bo_eff[128:]],
        axis=1).astype(np.float32))

    xT_all = [
        np.ascontiguousarray(
            np.concatenate([feat0[b].reshape(C, HW), feat1[b].reshape(C, HW)],
                           axis=1)).astype(np.float16)
        for b in range(B)
    ]

    in_maps = []
    for core in range(NCORES):
        b, g = core // 4, core % 4
        cols0 = slice(g * 1024, (g + 1) * 1024)
        cols1 = slice(HW + g * 1024, HW + (g + 1) * 1024)
        xTq = np.ascontiguousarray(
            np.concatenate([xT_all[b][:, cols0], xT_all[b][:, cols1]],
                           axis=1))
        in_maps.append({
            "xT": xT_all[b], "xTq": xTq,
            "wpack": wpack, "bpack": bpack,
        })

    global _last_in_maps
    _last_in_maps = in_maps

    nc = _get_compiled()
    res = run_bass_kernel_spmd(nc, in_maps, core_ids=list(range(NCORES)))

    full = np.empty((B, C, HW), dtype=np.float32)
    for core in range(NCORES):
        b, g = core // 4, core % 4
        full[b][:, g * 1024:(g + 1) * 1024] = res.results[core]["out"]
    return full.reshape(B, C, H, W)
